# revision 2
# baseline (speedup 1.0000x reference)
"""Trainium2 Bass kernel v2 for nn_Gtransformerblock (HAN-style 2-head graph
transformer block) on 8 NeuronCores.

Row-sharded (512 rows/core), one AllGather per head of the hidden features
x^T (f16, hid-major so no transposes are needed anywhere), with the QK^T
fold S = xa @ (M @ xa^T), M = Wqb Wkb^T precomputed on host.

Per core c (rows q = 512c..512c+511), per head i:
- hW = h @ W1 from full h (f32 matmul, both heads fused in the moving
  operand), split bf16 hi/lo per node-tile as [hi|lo] stationaries.
- x^T[64, 512]: stacked hi/lo accumulation vs adjT (bf16 moving), the two
  partition halves summed via an identity-stack matmul, then relu -> xaT
  (f32 + f16 copies, ones row at 64).
- AllGather xaT f16 [65, 512] -> xTa_full [65, 4096].
- Qt = M^T @ xaT32 (f32 matmul) -> f16.  S^T_j = xTa_j.T @ Qt (f16 x f16).
- eb = exp(S - 75) bf16 (unmasked; C=75 keeps garbage cols finite in f32),
  et = adjT * eb (bf16 2x elementwise on DVE/Pool), masked entries restored
  by the rank-1 correction e^{-C} * colsum(Va) folded into the P^T init.
- Va = [V | 1] via the Wvb1 e_64 column; P^T[65, 512] accumulates E@Va and
  Z (row 64) in one chain.
- logits = relu(P0)/Z0 @ predW + relu(P1)/Z1 @ predW + pred_b; row softmax.
"""
import sys

import numpy as np
import ml_dtypes

if "/opt/trn_rl_repo" not in sys.path:
    sys.path.insert(0, "/opt/trn_rl_repo")

import concourse.bass as bass
import concourse.mybir as mybir
import concourse.tile as tile
from concourse import bacc
from concourse import bass_utils

F32 = mybir.dt.float32
FP8 = mybir.dt.float8e4
F16 = mybir.dt.float16
BF16 = mybir.dt.bfloat16
AF = mybir.ActivationFunctionType
ALU = mybir.AluOpType
NPF16 = np.float16

W = 8          # cores
N = 4096       # nodes
R = N // W     # rows per core (512)
IN = 512
HID = 64
XA = HID + 1   # x plus ones row
H = 2
OUT = 8
C_SHIFT = 75.0  # global softmax shift; S<=~150 so exp(S-C) stays finite


def build():
    nc = bacc.Bacc("TRN2", target_bir_lowering=False, debug=False,
                   enable_asserts=False, num_devices=W)

    # partition-major adjT: [i, p, t, q] = adj[i, Rc+q, 128t+p], bf16
    adjT = nc.dram_tensor("adjT", [H, 128, 32, R], BF16, kind="ExternalInput")
    # hT chunks: [ec, p, n] = h[n, 128ec+p], bf16 hi + lo halves
    hThi = nc.dram_tensor("hThi", [4, 128, N], BF16, kind="ExternalInput")
    hTlo = nc.dram_tensor("hTlo", [4, 128, N], BF16, kind="ExternalInput")
    # fused W1: [p, ec, k] with k = 64*i + hid, bf16 hi + lo halves
    W1fhi = nc.dram_tensor("W1fhi", [128, 4, 2 * HID], BF16,
                           kind="ExternalInput")
    W1flo = nc.dram_tensor("W1flo", [128, 4, 2 * HID], BF16,
                           kind="ExternalInput")
    # M[i] = Wqb_i @ Wkb_i^T, packed [65, 2, 65] f32
    Mt = nc.dram_tensor("Mt", [XA, H, XA], F32, kind="ExternalInput")
    # Wvb1[i] = [[Wv; bv] | e_64], packed [65, 2, 65] f16
    Wvb1 = nc.dram_tensor("Wvb1", [XA, H, XA], F16, kind="ExternalInput")
    b1 = nc.dram_tensor("b1", [HID, H], F32, kind="ExternalInput")
    predWb = nc.dram_tensor("predWb", [XA, OUT], F32, kind="ExternalInput")
    Iden = nc.dram_tensor("Iden", [128, 128], F32, kind="ExternalInput")
    out = nc.dram_tensor("out", [R, OUT], F32, kind="ExternalOutput")

    with tile.TileContext(nc) as tc:
        with (
            tc.tile_pool(name="const", bufs=1) as const,
            tc.tile_pool(name="sbw", bufs=2) as sbw,
            tc.tile_pool(name="ps_s", bufs=2, space="PSUM") as ps_s,
            tc.tile_pool(name="ps_pt", bufs=2, space="PSUM") as ps_pt,
            tc.tile_pool(name="ps_small", bufs=2, space="PSUM") as ps_small,
            tc.tile_pool(name="dram", bufs=1, space="DRAM") as dram,
        ):
            # ---- constants ----
            W1fhi_t = const.tile([128, 4, 2 * HID], BF16, tag="W1fhi")
            nc.sync.dma_start(W1fhi_t[:], W1fhi[:, :, :])
            W1flo_t = const.tile([128, 4, 2 * HID], BF16, tag="W1flo")
            nc.sync.dma_start(W1flo_t[:], W1flo[:, :, :])
            M_t = const.tile([XA, H, XA], F32, tag="Mt")
            nc.sync.dma_start(M_t[:], Mt[:, :, :])
            Wv_t = const.tile([XA, H, XA], F16, tag="Wvb1")
            nc.sync.dma_start(Wv_t[:], Wvb1[:, :, :])
            b1_t = const.tile([HID, H], F32, tag="b1")
            nc.sync.dma_start(b1_t[:], b1[:, :])
            predWb_t = const.tile([XA, OUT], F32, tag="predWb")
            nc.sync.dma_start(predWb_t[:], predWb[:, :])
            ones_row = const.tile([1, 128], F32, tag="ones_row")
            nc.vector.memset(ones_row[:], 1.0)
            ones65 = const.tile([XA, 1], F32, tag="ones65")
            nc.vector.memset(ones65[:], 1.0)
            pb_row = const.tile([1, OUT], F32, tag="pb_row")
            nc.sync.dma_start(pb_row[:], predWb[HID:XA, :])
            neg_c = const.tile([128, 1], F32, tag="neg_c")
            nc.vector.memset(neg_c[:], -C_SHIFT)
            ones_col128 = const.tile([128, 1], BF16, tag="ones_col128")
            nc.vector.memset(ones_col128[:], 1.0)
            expC128 = const.tile([1, 128], BF16, tag="expC128")
            nc.vector.memset(expC128[:], float(np.exp(-C_SHIFT)))
            IdenT = const.tile([128, 128], F32, tag="IdenT")
            nc.sync.dma_start(IdenT[:], Iden[:, :])

            # xaT tiles pre-allocated; ones rows set off the critical path
            xaT32_pre, xaT16_pre = [], []
            for i in range(H):
                t32 = const.tile([XA, R], F32, tag=f"xaT32_{i}",
                                 name=f"xaT32p{i}")
                nc.vector.memset(t32[HID:XA, :], 1.0)
                xaT32_pre.append(t32)
                t16 = const.tile([XA, R], F16, tag=f"xaT16_{i}",
                                 name=f"xaT16p{i}")
                nc.vector.memset(t16[HID:XA, :], 1.0)
                xaT16_pre.append(t16)

            # ---- big streams: full hT (f32) then adjT (bf16) ----
            hThi_t, hTlo_t = [], []
            for ec in range(4):
                thi = const.tile([128, N], BF16, tag=f"hThi{ec}", name=f"thi{ec}")
                nc.gpsimd.dma_start(thi[:], hThi[ec, :, :])
                hThi_t.append(thi)
                tlo = const.tile([128, N], BF16, tag=f"hTlo{ec}", name=f"tlo{ec}")
                nc.gpsimd.dma_start(tlo[:], hTlo[ec, :, :])
                hTlo_t.append(tlo)
            adjt_t = [[None] * 4 for _ in range(H)]  # [i][chunk of 8 tiles]
            for i in range(H):
                for ch in range(4):
                    t = const.tile([128, 8 * R], BF16, tag=f"adjt{i}_{ch}")
                    if i == 0:
                        nc.gpsimd.dma_start(
                            t[:],
                            adjT[i, :, 8 * ch:8 * (ch + 1), :].rearrange(
                                "p t q -> p (t q)"))
                    else:
                        # half-chunks: halves the g_in0 write's queue wait on
                        # the DMA device
                        for hh in range(2):
                            nc.gpsimd.dma_start(
                                t[:, 4 * R * hh:4 * R * (hh + 1)],
                                adjT[i, :, 8 * ch + 4 * hh:
                                     8 * ch + 4 * (hh + 1), :].rearrange(
                                    "p t q -> p (t q)"))
                    adjt_t[i][ch] = t

            # ---- DRAM bounce buffers for the xaT gathers ----
            g_in = [dram.tile([XA, R], F16, name=f"g_in{i}", tag=f"g_in{i}")
                    for i in range(H)]
            g_out = [dram.tile([W, XA, R], F16, addr_space="Shared",
                               name=f"g_out{i}", tag=f"g_out{i}")
                     for i in range(H)]

            # ---- phase 1: hW (heads fused in moving) + bf16 hi/lo split ----
            # hilo group tiles hold BOTH heads in matmul layout:
            # cols [0 : 128*cnt) = hi of slots (t, h-interleaved 64s),
            # cols [128*cnt : 256*cnt) = lo. The x^T stationary de-interleaves
            # via a 2-chunk access pattern.
            hgroups = []  # (t0, cnt, hilo_tile)
            def hw_round(groups):
                for ec in range(4):
                    passes = [(hThi_t[ec], W1fhi_t), (hThi_t[ec], W1flo_t),
                              (hTlo_t[ec], W1fhi_t)]
                    for pi, (ht, wt) in enumerate(passes):
                        for ps_t, t0, cnt, hl in groups:
                            for u in range(cnt):
                                tt = t0 + u
                                # start=True zeroes the whole PSUM bank on
                                # HW: exactly one start per 2KB bank
                                nc.tensor.matmul(
                                    ps_t[:, 128 * u:128 * (u + 1)],
                                    ht[:, 128 * tt:128 * (tt + 1)],
                                    wt[:, ec, :],
                                    start=(pi == 0 and ec == 0 and u % 4 == 0),
                                    stop=(pi == 2 and ec == 3),
                                    skip_group_check=True)
                for ps_t, t0, cnt, hl in groups:
                    w = 128 * cnt
                    nc.scalar.activation(hl[:, 0:w], ps_t[:], AF.Copy)
                    nc.vector.tensor_tensor(hl[:, w:2 * w], ps_t[:],
                                            hl[:, 0:w], op=ALU.subtract)
                    hgroups.append((t0, cnt, hl))

            gAll = [
                (ps_s.tile([128, 512], F32, tag="ps_s", bufs=4,
                           name="ps_hw0"), 0, 4,
                 const.tile([128, 1024], BF16, tag="hg0", name="hg0")),
                (ps_s.tile([128, 512], F32, tag="ps_s", bufs=4,
                           name="ps_hw1"), 4, 4,
                 const.tile([128, 1024], BF16, tag="hg1", name="hg1")),
                (ps_s.tile([128, 512], F32, tag="ps_s", bufs=4,
                           name="ps_hw2"), 8, 4,
                 const.tile([128, 1024], BF16, tag="hg2", name="hg2")),
                (ps_s.tile([128, 512], F32, tag="ps_s", bufs=4,
                           name="ps_hw3"), 12, 4,
                 const.tile([128, 1024], BF16, tag="hg3", name="hg3")),
                (ps_small.tile([128, 512], F32, tag="ps_small",
                               name="ps_hw4"), 16, 4,
                 const.tile([128, 1024], BF16, tag="hg4", name="hg4")),
                (ps_small.tile([128, 512], F32, tag="ps_small",
                               name="ps_hw5"), 20, 4,
                 const.tile([128, 1024], BF16, tag="hg5", name="hg5")),
                (ps_pt.tile([128, 512], F32, tag="ps_pt", name="ps_hw6"),
                 24, 4,
                 const.tile([128, 1024], BF16, tag="hg6", name="hg6")),
                (ps_pt.tile([128, 512], F32, tag="ps_pt", name="ps_hw7"),
                 28, 4,
                 const.tile([128, 1024], BF16, tag="hg7", name="hg7")),
            ]
            hw_round(gAll)
            hgroups.sort()

            def hilo_stat(i, j, half):
                # hi (half=0) or lo (half=1) stationary [128, 64]
                for t0, cnt, hl in hgroups:
                    if t0 <= j < t0 + cnt:
                        u = j - t0
                        off = 128 * cnt * half + 128 * u + HID * i
                        return hl[:, off:off + HID]
                raise AssertionError(j)

            # ---- phase 2 per head: x^T accum -> xaT -> Qt -> gather ----
            xaT32_t, Qt_t = [], []
            for i in range(H):
                ps_xt = ps_small.tile([HID, R], F32, tag="ps_small",
                                      name=f"ps_xt{i}")
                for j in range(32):
                    av = adjt_t[i][j // 8][:, R * (j % 8):R * (j % 8 + 1)]
                    nc.tensor.matmul(ps_xt[:], hilo_stat(i, j, 0), av,
                                     start=(j == 0), stop=False,
                                     skip_group_check=True)
                    nc.tensor.matmul(ps_xt[:], hilo_stat(i, j, 1), av,
                                     start=False, stop=(j == 31),
                                     skip_group_check=True)
                # x^T combined in PSUM: f16 and f32 relu copies in parallel
                xaT32, xaT16 = xaT32_pre[i], xaT16_pre[i]
                nc.vector.tensor_scalar(xaT16[0:HID, :], ps_xt[:],
                                        b1_t[:, i:i + 1], 0.0,
                                        op0=ALU.add, op1=ALU.max)
                nc.scalar.activation(xaT32[0:HID, :], ps_xt[:], AF.Relu,
                                     bias=b1_t[:, i:i + 1])
                xaT32_t.append(xaT32)
                # ship xaT16 and gather
                nc.sync.dma_start(g_in[i][:], xaT16[:])
                nc.gpsimd.collective_compute(
                    "AllGather", ALU.bypass, replica_groups=[list(range(W))],
                    ins=[g_in[i][:].opt()], outs=[g_out[i][:].opt()])
                # Qt = M^T @ xaT32 (f32), rounded to f16
                ps_q = ps_small.tile([XA, R], F32, tag="ps_small",
                                     name=f"ps_q{i}")
                nc.tensor.matmul(ps_q[:], M_t[:, i, :], xaT32[:],
                                 start=True, stop=True)
                Qt16 = const.tile([XA, R], F16, tag=f"Qt16_{i}")
                nc.vector.tensor_copy(Qt16[:], ps_q[:])
                Qt_t.append(Qt16)

            # ---- phase 3 per head: read gather, Va, cs, S/E/P pipeline ----
            rp_t = []
            for i in range(H):
                xTa = const.tile([XA, N], F16, tag=f"xTa_{i}")
                xTa_v = xTa[:].rearrange("p (w n) -> p w n", w=W)
                g_v = g_out[i][:].rearrange("w p n -> p w n")
                nc.sync.dma_start(xTa_v[:, 0:W // 2, :], g_v[:, 0:W // 2, :])
                nc.sync.dma_start(xTa_v[:, W // 2:W, :], g_v[:, W // 2:W, :])
                # Va tiles vf_sb [128, 32*65] bf16, interleaved with the
                # S/E/P pipeline; cs + masked-entry correction added at end
                vf_sb = const.tile([128, 32 * XA], BF16, tag=f"vf_{i}")
                ps_P = ps_pt.tile([128, 4 * XA], F32, tag="ps_pt",
                                  name=f"ps_P{i}")
                ps_cs = ps_pt.tile([1, XA], F32, tag="ps_pt",
                                   name=f"ps_cs{i}")
                def emit_P(j, et):
                    for s in range(4):
                        nc.tensor.matmul(
                            ps_P[:, XA * s:XA * (s + 1)],
                            et[:, 128 * s:128 * (s + 1)],
                            vf_sb[:, XA * j:XA * (j + 1)],
                            start=(j == 0 and s == 0), stop=False,
                            skip_group_check=True)

                et_prev = None
                for j in range(32):  # per-j pipeline, depth 4; P delayed one
                    # step so the PE queue never waits on the current mask
                    if j % 4 == 0:   # produce vf for j4-group j//4
                        j4 = j // 4
                        ps_v = ps_small.tile([128, 4 * XA], F32,
                                             tag="ps_small",
                                             name=f"ps_v{i}{j4}")
                        for jj in range(4):
                            jv = 4 * j4 + jj
                            nc.tensor.matmul(
                                ps_v[:, XA * jj:XA * (jj + 1)],
                                xTa[:, 128 * jv:128 * (jv + 1)],
                                Wv_t[:, i, :], start=(jj == 0), stop=True,
                                skip_group_check=True)
                        nc.vector.tensor_copy(
                            vf_sb[:, 4 * XA * j4:4 * XA * (j4 + 1)], ps_v[:])
                        for jj in range(4):
                            jv = 4 * j4 + jj
                            nc.tensor.matmul(
                                ps_cs[:], ones_col128[:],
                                vf_sb[:, XA * jv:XA * (jv + 1)],
                                start=(jv == 0), stop=(jv == 31),
                                skip_group_check=True)
                    ps_sp = ps_s.tile([128, 512], F32, tag="ps_s", bufs=4,
                                      name=f"ps_sp{i}{j}")
                    nc.tensor.matmul(ps_sp[:],
                                     xTa[:, 128 * j:128 * (j + 1)],
                                     Qt_t[i][:], start=True, stop=True)
                    eb = sbw.tile([128, 512], BF16, tag="eb", bufs=6,
                                  name=f"eb{i}{j}")
                    nc.scalar.activation(eb[:], ps_sp[:], AF.Exp,
                                         bias=neg_c[:])
                    et = sbw.tile([128, 512], BF16, tag="et", bufs=6,
                                  name=f"et{i}{j}")
                    adj_v = adjt_t[i][j // 8][:, 512 * (j % 8):512 * (j % 8 + 1)]
                    eng = nc.vector
                    eng.tensor_tensor(et[:], adj_v, eb[:], op=ALU.mult)
                    if et_prev is not None:
                        emit_P(j - 1, et_prev)
                    et_prev = et
                emit_P(31, et_prev)
                # end-correction e^{-C} * cs per slice (cs accumulated
                # inside the loop above)
                cs_sb = sbw.tile([1, XA], BF16, tag="cs", name=f"cs{i}")
                nc.vector.tensor_copy(cs_sb[:], ps_cs[:])
                for s in range(4):
                    nc.tensor.matmul(ps_P[:, XA * s:XA * (s + 1)],
                                     expC128[:], cs_sb[:],
                                     start=False, stop=True,
                                     skip_group_check=True)
                # relu, then transpose the four [128, 65] slices into rp
                u_sb = sbw.tile([128, 4 * XA], F32, tag="u_sb", name=f"u_sb{i}")
                nc.scalar.activation(u_sb[:], ps_P[:], AF.Relu)
                rp = const.tile([XA, R], F32, tag=f"rp{i}")
                for s in range(4):
                    ps_tr = ps_small.tile([XA, 128], F32, tag="ps_small",
                                          name=f"ps_tr{i}{s}")
                    nc.tensor.transpose(ps_tr[:], u_sb[:, XA * s:XA * (s + 1)],
                                        IdenT[:])
                    nc.vector.tensor_copy(rp[:, 128 * s:128 * (s + 1)],
                                          ps_tr[:])
                rp_t.append(rp)
                if i == 0:
                    # head-0 half of the logits tail, scheduled under the
                    # head-1 S-phase
                    h0_parts = []
                    for rc in range(4):
                        ps_za = ps_small.tile([128, 1], F32, tag="ps_small",
                                              name=f"ps_za{rc}")
                        nc.tensor.matmul(
                            ps_za[:], rp[HID:XA, 128 * rc:128 * (rc + 1)],
                            ones65[HID:XA, :], start=True, stop=True)
                        rz0 = sbw.tile([128, 1], F32, tag="sm", bufs=8,
                                       name=f"rz0h{rc}")
                        nc.vector.reciprocal(rz0[:], ps_za[:])
                        ps_la = ps_small.tile([128, OUT], F32,
                                              tag="ps_small",
                                              name=f"ps_la{rc}")
                        nc.tensor.matmul(
                            ps_la[:], rp[0:HID, 128 * rc:128 * (rc + 1)],
                            predWb_t[0:HID, :], start=True, stop=True)
                        u = sbw.tile([128, OUT], F32, tag="finh", bufs=4,
                                     name=f"uh{rc}")
                        nc.vector.tensor_scalar(u[:], ps_la[:], rz0[:],
                                                None, op0=ALU.mult)
                        h0_parts.append(u)

            # ---- phase 4: logits + final softmax ----
            ps_pb = ps_small.tile([128, OUT], F32, tag="ps_small", name="ps_pb")
            nc.tensor.matmul(ps_pb[:], ones_row[:], pb_row[:],
                             start=True, stop=True)
            pb_b = sbw.tile([128, OUT], F32, tag="pb_b")
            nc.vector.tensor_copy(pb_b[:], ps_pb[:])
            for rc in range(4):
                ps_z = ps_pt.tile([128, 1], F32, tag="ps_pt", name=f"ps_z{rc}")
                nc.tensor.matmul(ps_z[:],
                                 rp_t[1][HID:XA, 128 * rc:128 * (rc + 1)],
                                 ones65[HID:XA, :], start=True, stop=True)
                rz = sbw.tile([128, 1], F32, tag="sm", bufs=8, name=f"rz{rc}")
                nc.vector.reciprocal(rz[:], ps_z[:])
                ps_l1 = ps_small.tile([128, OUT], F32, tag="ps_small",
                                      name=f"ps_l1{rc}")
                nc.tensor.matmul(ps_l1[:],
                                 rp_t[1][0:HID, 128 * rc:128 * (rc + 1)],
                                 predWb_t[0:HID, :], start=True, stop=True)
                v = sbw.tile([128, OUT], F32, tag="fin", bufs=4, name=f"v{rc}")
                nc.vector.tensor_scalar(v[:], ps_l1[:], rz[:], None,
                                        op0=ALU.mult)
                w_ = sbw.tile([128, OUT], F32, tag="fin", bufs=4, name=f"w{rc}")
                nc.vector.tensor_tensor(w_[:], h0_parts[rc][:], v[:],
                                        op=ALU.add)
                wb = sbw.tile([128, OUT], F32, tag="fin", bufs=4, name=f"wb{rc}")
                nc.vector.tensor_tensor(wb[:], w_[:], pb_b[:], op=ALU.add)
                e = sbw.tile([128, OUT], F32, tag="fin", bufs=4, name=f"e{rc}")
                nc.scalar.activation(e[:], wb[:], AF.Exp)
                s = sbw.tile([128, 1], F32, tag="sm", bufs=8, name=f"s{rc}")
                nc.vector.reduce_sum(s[:], e[:], axis=mybir.AxisListType.X)
                rs = sbw.tile([128, 1], F32, tag="sm", bufs=8, name=f"rs{rc}")
                nc.vector.reciprocal(rs[:], s[:])
                o = sbw.tile([128, OUT], F32, tag="fin", bufs=4, name=f"o{rc}")
                nc.vector.tensor_scalar(o[:], e[:], rs[:], None, op0=ALU.mult)
                nc.sync.dma_start(out[128 * rc:128 * (rc + 1), :], o[:])

    nc.finalize()
    return nc


_NC = None


def _get_nc():
    global _NC
    if _NC is None:
        _NC = build()
    return _NC


def _prepare_in_maps(inputs):
    h = np.asarray(inputs["h"], np.float32)
    adj = np.asarray(inputs["adj"], np.float32)
    W1v = np.asarray(inputs["W1"], np.float32)
    b1v = np.asarray(inputs["b1"], np.float32)

    # shared (replicated) tensors
    hTv = np.ascontiguousarray(h.T.reshape(4, 128, N))
    hThi = hTv.astype(ml_dtypes.bfloat16)
    hTlo = (hTv - hThi.astype(np.float32)).astype(ml_dtypes.bfloat16)
    # W1f[p, ec, 64i+k] = W1[i, 128ec+p, k]
    W1f = np.ascontiguousarray(
        W1v.reshape(H, 4, 128, HID).transpose(2, 1, 0, 3).reshape(128, 4, 2 * HID))
    W1fhi = W1f.astype(ml_dtypes.bfloat16)
    W1flo = (W1f - W1fhi.astype(np.float32)).astype(ml_dtypes.bfloat16)
    b1c = np.ascontiguousarray(b1v.T)  # [64, 2]
    Mt = np.zeros((XA, H, XA), np.float32)
    Wvb1 = np.zeros((XA, H, XA), np.float32)
    for i in range(H):
        Wqb = np.concatenate([np.asarray(inputs["Wq"], np.float32)[i],
                              np.asarray(inputs["bq"], np.float32)[i][None]], 0)
        Wkb = np.concatenate([np.asarray(inputs["Wk"], np.float32)[i],
                              np.asarray(inputs["bk"], np.float32)[i][None]], 0)
        Mt[:, i, :] = Wqb @ Wkb.T
        Wvb = np.concatenate([np.asarray(inputs["Wv"], np.float32)[i],
                              np.asarray(inputs["bv"], np.float32)[i][None]], 0)
        Wvb1[:, i, 0:HID] = Wvb[:, 0:HID]
        Wvb1[HID, i, HID] = 1.0
    Wvb1 = Wvb1.astype(NPF16)
    predWbv = np.ascontiguousarray(np.concatenate(
        [np.asarray(inputs["pred_W"], np.float32),
         np.asarray(inputs["pred_b"], np.float32)[None, :]], axis=0))
    Iden = np.eye(128, dtype=np.float32)

    in_maps = []
    for c in range(W):
        rows = slice(R * c, R * (c + 1))
        # adjTpm[i, p, t, q] = adj[i, Rc+q, 128t+p]
        a = adj[:, rows, :]                       # [H, R, N]
        a = a.transpose(0, 2, 1)                  # [H, N, R] (n, q)
        a = a.reshape(H, 32, 128, R)              # [H, t, p, q]
        a = np.ascontiguousarray(a.transpose(0, 2, 1, 3))  # [H, p, t, q]
        in_maps.append(dict(adjT=a.astype(ml_dtypes.bfloat16),
                            hThi=hThi, hTlo=hTlo, W1fhi=W1fhi, W1flo=W1flo,
                            Mt=Mt, Wvb1=Wvb1, b1=b1c,
                            predWb=predWbv, Iden=Iden))
    return in_maps


def run(inputs, **run_kwargs):
    nc = _get_nc()
    in_maps = _prepare_in_maps(inputs)
    res = bass_utils.run_bass_kernel_spmd(nc, in_maps, core_ids=list(range(W)),
                                          **run_kwargs)
    outp = np.concatenate([res.results[c]["out"] for c in range(W)], axis=0)
    return outp, res


def kernel(**inputs) -> np.ndarray:
    outp, _ = run(inputs)
    return outp


# revision 3
# speedup vs baseline: 1.0383x; 1.0383x over previous
"""Trainium2 Bass kernel v2 for nn_Gtransformerblock (HAN-style 2-head graph
transformer block) on 8 NeuronCores.

Row-sharded (512 rows/core), one AllGather per head of the hidden features
x^T (f16, hid-major so no transposes are needed anywhere), with the QK^T
fold S = xa @ (M @ xa^T), M = Wqb Wkb^T precomputed on host.

Per core c (rows q = 512c..512c+511), per head i:
- hW = h @ W1 from full h (f32 matmul, both heads fused in the moving
  operand), split bf16 hi/lo per node-tile as [hi|lo] stationaries.
- x^T[64, 512]: stacked hi/lo accumulation vs adjT (bf16 moving), the two
  partition halves summed via an identity-stack matmul, then relu -> xaT
  (f32 + f16 copies, ones row at 64).
- AllGather xaT f16 [65, 512] -> xTa_full [65, 4096].
- Qt = M^T @ xaT32 (f32 matmul) -> f16.  S^T_j = xTa_j.T @ Qt (f16 x f16).
- eb = exp(S - 75) bf16 (unmasked; C=75 keeps garbage cols finite in f32),
  et = adjT * eb (bf16 2x elementwise on DVE/Pool), masked entries restored
  by the rank-1 correction e^{-C} * colsum(Va) folded into the P^T init.
- Va = [V | 1] via the Wvb1 e_64 column; P^T[65, 512] accumulates E@Va and
  Z (row 64) in one chain.
- logits = relu(P0)/Z0 @ predW + relu(P1)/Z1 @ predW + pred_b; row softmax.
"""
import sys

import numpy as np
import ml_dtypes

if "/opt/trn_rl_repo" not in sys.path:
    sys.path.insert(0, "/opt/trn_rl_repo")

import concourse.bass as bass
import concourse.mybir as mybir
import concourse.tile as tile
from concourse import bacc
from concourse import bass_utils

F32 = mybir.dt.float32
FP8 = mybir.dt.float8e4
F16 = mybir.dt.float16
BF16 = mybir.dt.bfloat16
AF = mybir.ActivationFunctionType
ALU = mybir.AluOpType
NPF16 = np.float16

W = 8          # cores
N = 4096       # nodes
R = N // W     # rows per core (512)
IN = 512
HID = 64
XA = HID + 1   # x plus ones row
H = 2
OUT = 8
C_SHIFT = 75.0  # global softmax shift; S<=~150 so exp(S-C) stays finite


def build():
    nc = bacc.Bacc("TRN2", target_bir_lowering=False, debug=False,
                   enable_asserts=False, num_devices=W)

    # partition-major adjT: [i, p, t, q] = adj[i, Rc+q, 128t+p], bf16
    adjT = nc.dram_tensor("adjT", [H, 128, 32, R], BF16, kind="ExternalInput")
    # hT chunks: [ec, p, n] = h[n, 128ec+p], bf16 hi + lo halves
    hThi = nc.dram_tensor("hThi", [4, 128, N], BF16, kind="ExternalInput")
    hTlo = nc.dram_tensor("hTlo", [4, 128, N], BF16, kind="ExternalInput")
    # fused W1: [p, ec, k] with k = 64*i + hid, bf16 hi + lo halves
    W1fhi = nc.dram_tensor("W1fhi", [128, 4, 2 * HID], BF16,
                           kind="ExternalInput")
    W1flo = nc.dram_tensor("W1flo", [128, 4, 2 * HID], BF16,
                           kind="ExternalInput")
    # M[i] = Wqb_i @ Wkb_i^T, packed [65, 2, 65] f32
    Mt = nc.dram_tensor("Mt", [XA, H, XA], F32, kind="ExternalInput")
    # Wvb1[i] = [[Wv; bv] | e_64], packed [65, 2, 65] f16
    Wvb1 = nc.dram_tensor("Wvb1", [XA, H, XA], F16, kind="ExternalInput")
    b1 = nc.dram_tensor("b1", [HID, H], F32, kind="ExternalInput")
    predWb = nc.dram_tensor("predWb", [XA, OUT], F32, kind="ExternalInput")
    Iden = nc.dram_tensor("Iden", [128, 128], F32, kind="ExternalInput")
    out = nc.dram_tensor("out", [R, OUT], F32, kind="ExternalOutput")

    with tile.TileContext(nc) as tc:
        with (
            tc.tile_pool(name="const", bufs=1) as const,
            tc.tile_pool(name="sbw", bufs=2) as sbw,
            tc.tile_pool(name="ps_s", bufs=2, space="PSUM") as ps_s,
            tc.tile_pool(name="ps_pt", bufs=2, space="PSUM") as ps_pt,
            tc.tile_pool(name="ps_small", bufs=2, space="PSUM") as ps_small,
            tc.tile_pool(name="dram", bufs=1, space="DRAM") as dram,
        ):
            # ---- constants ----
            W1fhi_t = const.tile([128, 4, 2 * HID], BF16, tag="W1fhi")
            nc.sync.dma_start(W1fhi_t[:], W1fhi[:, :, :])
            W1flo_t = const.tile([128, 4, 2 * HID], BF16, tag="W1flo")
            nc.sync.dma_start(W1flo_t[:], W1flo[:, :, :])
            M_t = const.tile([XA, H, XA], F32, tag="Mt")
            nc.sync.dma_start(M_t[:], Mt[:, :, :])
            Wv_t = const.tile([XA, H, XA], F16, tag="Wvb1")
            nc.sync.dma_start(Wv_t[:], Wvb1[:, :, :])
            b1_t = const.tile([HID, H], F32, tag="b1")
            nc.sync.dma_start(b1_t[:], b1[:, :])
            predWb_t = const.tile([XA, OUT], F32, tag="predWb")
            nc.sync.dma_start(predWb_t[:], predWb[:, :])
            ones_row = const.tile([1, 128], F32, tag="ones_row")
            nc.vector.memset(ones_row[:], 1.0)
            ones65 = const.tile([XA, 1], F32, tag="ones65")
            nc.vector.memset(ones65[:], 1.0)
            pb_row = const.tile([1, OUT], F32, tag="pb_row")
            nc.sync.dma_start(pb_row[:], predWb[HID:XA, :])
            neg_c = const.tile([128, 1], F32, tag="neg_c")
            nc.vector.memset(neg_c[:], -C_SHIFT)
            ones_col128 = const.tile([128, 1], BF16, tag="ones_col128")
            nc.vector.memset(ones_col128[:], 1.0)
            expC128 = const.tile([1, 128], BF16, tag="expC128")
            nc.vector.memset(expC128[:], float(np.exp(-C_SHIFT)))
            IdenT = const.tile([128, 128], F32, tag="IdenT")
            nc.sync.dma_start(IdenT[:], Iden[:, :])

            # xaT tiles pre-allocated; ones rows set off the critical path
            xaT32_pre, xaT16_pre = [], []
            for i in range(H):
                t32 = const.tile([XA, R], F32, tag=f"xaT32_{i}",
                                 name=f"xaT32p{i}")
                nc.vector.memset(t32[HID:XA, :], 1.0)
                xaT32_pre.append(t32)
                t16 = const.tile([XA, R], F16, tag=f"xaT16_{i}",
                                 name=f"xaT16p{i}")
                nc.vector.memset(t16[HID:XA, :], 1.0)
                xaT16_pre.append(t16)

            # ---- big streams: full hT (f32) then adjT (bf16) ----
            hThi_t, hTlo_t = [], []
            for ec in range(4):
                thi = const.tile([128, N], BF16, tag=f"hThi{ec}", name=f"thi{ec}")
                nc.gpsimd.dma_start(thi[:], hThi[ec, :, :])
                hThi_t.append(thi)
                tlo = const.tile([128, N], BF16, tag=f"hTlo{ec}", name=f"tlo{ec}")
                nc.gpsimd.dma_start(tlo[:], hTlo[ec, :, :])
                hTlo_t.append(tlo)
            adjt_t = [[None] * 4 for _ in range(H)]  # [i][chunk of 8 tiles]
            for i in range(H):
                for ch in range(4):
                    t = const.tile([128, 8 * R], BF16, tag=f"adjt{i}_{ch}")
                    if i == 0:
                        nc.gpsimd.dma_start(
                            t[:],
                            adjT[i, :, 8 * ch:8 * (ch + 1), :].rearrange(
                                "p t q -> p (t q)"))
                    else:
                        # quarter-chunks: shortens the g_in0 write's queue
                        # wait on the DMA device
                        for hh in range(4):
                            nc.gpsimd.dma_start(
                                t[:, 2 * R * hh:2 * R * (hh + 1)],
                                adjT[i, :, 8 * ch + 2 * hh:
                                     8 * ch + 2 * (hh + 1), :].rearrange(
                                    "p t q -> p (t q)"))
                    adjt_t[i][ch] = t

            # ---- DRAM bounce buffers for the xaT gathers ----
            g_in = [dram.tile([XA, R], F16, name=f"g_in{i}", tag=f"g_in{i}")
                    for i in range(H)]
            g_out = [dram.tile([W, XA, R], F16, addr_space="Shared",
                               name=f"g_out{i}", tag=f"g_out{i}")
                     for i in range(H)]

            # ---- phase 1: hW (heads fused in moving) + bf16 hi/lo split ----
            # hilo group tiles hold BOTH heads in matmul layout:
            # cols [0 : 128*cnt) = hi of slots (t, h-interleaved 64s),
            # cols [128*cnt : 256*cnt) = lo. The x^T stationary de-interleaves
            # via a 2-chunk access pattern.
            hgroups = []  # (t0, cnt, hilo_tile)
            def hw_round(groups):
                for ec in range(4):
                    passes = [(hThi_t[ec], W1fhi_t), (hThi_t[ec], W1flo_t),
                              (hTlo_t[ec], W1fhi_t)]
                    for pi, (ht, wt) in enumerate(passes):
                        for ps_t, t0, cnt, hl in groups:
                            for u in range(cnt):
                                tt = t0 + u
                                # start=True zeroes the whole PSUM bank on
                                # HW: exactly one start per 2KB bank
                                nc.tensor.matmul(
                                    ps_t[:, 128 * u:128 * (u + 1)],
                                    ht[:, 128 * tt:128 * (tt + 1)],
                                    wt[:, ec, :],
                                    start=(pi == 0 and ec == 0 and u % 4 == 0),
                                    stop=(pi == 2 and ec == 3),
                                    skip_group_check=True)
                for ps_t, t0, cnt, hl in groups:
                    w = 128 * cnt
                    nc.scalar.activation(hl[:, 0:w], ps_t[:], AF.Copy)
                    nc.vector.tensor_tensor(hl[:, w:2 * w], ps_t[:],
                                            hl[:, 0:w], op=ALU.subtract)
                    hgroups.append((t0, cnt, hl))

            gAll = [
                (ps_s.tile([128, 1024], F32, tag="ps_s", bufs=2,
                           name="ps_hw0"), 0, 8,
                 const.tile([128, 2048], BF16, tag="hg0", name="hg0")),
                (ps_s.tile([128, 1024], F32, tag="ps_s", bufs=2,
                           name="ps_hw1"), 8, 8,
                 const.tile([128, 2048], BF16, tag="hg1", name="hg1")),
                (ps_small.tile([128, 512], F32, tag="ps_small",
                               name="ps_hw4"), 16, 4,
                 const.tile([128, 1024], BF16, tag="hg4", name="hg4")),
                (ps_small.tile([128, 512], F32, tag="ps_small",
                               name="ps_hw5"), 20, 4,
                 const.tile([128, 1024], BF16, tag="hg5", name="hg5")),
                (ps_pt.tile([128, 512], F32, tag="ps_pt", name="ps_hw6"),
                 24, 4,
                 const.tile([128, 1024], BF16, tag="hg6", name="hg6")),
                (ps_pt.tile([128, 512], F32, tag="ps_pt", name="ps_hw7"),
                 28, 4,
                 const.tile([128, 1024], BF16, tag="hg7", name="hg7")),
            ]
            hw_round(gAll)
            hgroups.sort()

            def hilo_stat(i, j, half):
                # hi (half=0) or lo (half=1) stationary [128, 64]
                for t0, cnt, hl in hgroups:
                    if t0 <= j < t0 + cnt:
                        u = j - t0
                        off = 128 * cnt * half + 128 * u + HID * i
                        return hl[:, off:off + HID]
                raise AssertionError(j)

            # ---- phase 2 per head: x^T accum -> xaT -> Qt -> gather ----
            xaT32_t, Qt_t = [], []
            for i in range(H):
                ps_xt = ps_small.tile([HID, R], F32, tag="ps_small",
                                      name=f"ps_xt{i}")
                for j in range(32):
                    av = adjt_t[i][j // 8][:, R * (j % 8):R * (j % 8 + 1)]
                    nc.tensor.matmul(ps_xt[:], hilo_stat(i, j, 0), av,
                                     start=(j == 0), stop=False,
                                     skip_group_check=True)
                    nc.tensor.matmul(ps_xt[:], hilo_stat(i, j, 1), av,
                                     start=False, stop=(j == 31),
                                     skip_group_check=True)
                # x^T combined in PSUM: f16 and f32 relu copies in parallel
                xaT32, xaT16 = xaT32_pre[i], xaT16_pre[i]
                nc.vector.tensor_scalar(xaT16[0:HID, :], ps_xt[:],
                                        b1_t[:, i:i + 1], 0.0,
                                        op0=ALU.add, op1=ALU.max)
                nc.scalar.activation(xaT32[0:HID, :], ps_xt[:], AF.Relu,
                                     bias=b1_t[:, i:i + 1])
                xaT32_t.append(xaT32)
                # ship xaT16 and gather
                nc.sync.dma_start(g_in[i][:], xaT16[:])
                nc.gpsimd.collective_compute(
                    "AllGather", ALU.bypass, replica_groups=[list(range(W))],
                    ins=[g_in[i][:].opt()], outs=[g_out[i][:].opt()])
                # Qt = M^T @ xaT32 (f32), rounded to f16
                ps_q = ps_small.tile([XA, R], F32, tag="ps_small",
                                     name=f"ps_q{i}")
                nc.tensor.matmul(ps_q[:], M_t[:, i, :], xaT32[:],
                                 start=True, stop=True)
                Qt16 = const.tile([XA, R], F16, tag=f"Qt16_{i}")
                nc.vector.tensor_copy(Qt16[:], ps_q[:])
                Qt_t.append(Qt16)

            # ---- phase 3 per head: read gather, Va, cs, S/E/P pipeline ----
            rp_t = []
            for i in range(H):
                xTa = const.tile([XA, N], F16, tag=f"xTa_{i}")
                xTa_v = xTa[:].rearrange("p (w n) -> p w n", w=W)
                g_v = g_out[i][:].rearrange("w p n -> p w n")
                nc.sync.dma_start(xTa_v[:, 0:W // 2, :], g_v[:, 0:W // 2, :])
                nc.sync.dma_start(xTa_v[:, W // 2:W, :], g_v[:, W // 2:W, :])
                # Va tiles vf_sb [128, 32*65] bf16, interleaved with the
                # S/E/P pipeline; cs + masked-entry correction added at end
                vf_sb = const.tile([128, 32 * XA], BF16, tag=f"vf_{i}")
                ps_P = ps_pt.tile([128, 4 * XA], F32, tag="ps_pt",
                                  name=f"ps_P{i}")
                ps_cs = ps_pt.tile([1, XA], F32, tag="ps_pt",
                                   name=f"ps_cs{i}")
                def emit_P(j, et):
                    for s in range(4):
                        nc.tensor.matmul(
                            ps_P[:, XA * s:XA * (s + 1)],
                            et[:, 128 * s:128 * (s + 1)],
                            vf_sb[:, XA * j:XA * (j + 1)],
                            start=(j == 0 and s == 0), stop=False,
                            skip_group_check=True)

                et_prev = None
                for j in range(32):  # per-j pipeline, depth 4; P delayed one
                    # step so the PE queue never waits on the current mask
                    if j % 4 == 0:   # produce vf for j4-group j//4
                        j4 = j // 4
                        ps_v = ps_small.tile([128, 4 * XA], F32,
                                             tag="ps_small",
                                             name=f"ps_v{i}{j4}")
                        for jj in range(4):
                            jv = 4 * j4 + jj
                            nc.tensor.matmul(
                                ps_v[:, XA * jj:XA * (jj + 1)],
                                xTa[:, 128 * jv:128 * (jv + 1)],
                                Wv_t[:, i, :], start=(jj == 0), stop=True,
                                skip_group_check=True)
                        nc.vector.tensor_copy(
                            vf_sb[:, 4 * XA * j4:4 * XA * (j4 + 1)], ps_v[:])
                        for jj in range(4):
                            jv = 4 * j4 + jj
                            nc.tensor.matmul(
                                ps_cs[:], ones_col128[:],
                                vf_sb[:, XA * jv:XA * (jv + 1)],
                                start=(jv == 0), stop=(jv == 31),
                                skip_group_check=True)
                    if j % 2 == 0:
                        ps_sp2 = ps_s.tile([128, 1024], F32, tag="ps_s",
                                           bufs=2, name=f"ps_sp{i}{j}")
                        eb2 = sbw.tile([128, 1024], BF16, tag="eb", bufs=4,
                                       name=f"eb{i}{j}")
                    ps_sp = ps_sp2[:, 512 * (j % 2):512 * (j % 2 + 1)]
                    nc.tensor.matmul(ps_sp,
                                     xTa[:, 128 * j:128 * (j + 1)],
                                     Qt_t[i][:], start=True, stop=True)
                    if j % 2 == 1:
                        nc.scalar.activation(eb2[:], ps_sp2[:], AF.Exp,
                                             bias=neg_c[:])
                    eb = eb2[:, 512 * (j % 2):512 * (j % 2 + 1)]
                    et = sbw.tile([128, 512], BF16, tag="et", bufs=6,
                                  name=f"et{i}{j}")
                    adj_v = adjt_t[i][j // 8][:, 512 * (j % 8):512 * (j % 8 + 1)]
                    if j % 2 == 1:
                        nc.vector.tensor_tensor(
                            et_prev2[:], adjt_t[i][(j - 1) // 8]
                            [:, 512 * ((j - 1) % 8):512 * ((j - 1) % 8 + 1)],
                            eb2[:, 0:512], op=ALU.mult)
                        nc.vector.tensor_tensor(et[:], adj_v,
                                                eb2[:, 512:1024], op=ALU.mult)
                    else:
                        et_prev2 = et
                    if et_prev is not None:
                        emit_P(j - 1, et_prev)
                    et_prev = et
                emit_P(31, et_prev)
                # end-correction e^{-C} * cs per slice (cs accumulated
                # inside the loop above)
                cs_sb = sbw.tile([1, XA], BF16, tag="cs", name=f"cs{i}")
                nc.vector.tensor_copy(cs_sb[:], ps_cs[:])
                for s in range(4):
                    nc.tensor.matmul(ps_P[:, XA * s:XA * (s + 1)],
                                     expC128[:], cs_sb[:],
                                     start=False, stop=True,
                                     skip_group_check=True)
                # relu, then transpose the four [128, 65] slices into rp
                u_sb = sbw.tile([128, 4 * XA], F32, tag="u_sb", name=f"u_sb{i}")
                nc.scalar.activation(u_sb[:], ps_P[:], AF.Relu)
                rp = const.tile([XA, R], F32, tag=f"rp{i}")
                for s in range(4):
                    ps_tr = ps_small.tile([XA, 128], F32, tag="ps_small",
                                          name=f"ps_tr{i}{s}")
                    nc.tensor.transpose(ps_tr[:], u_sb[:, XA * s:XA * (s + 1)],
                                        IdenT[:])
                    nc.vector.tensor_copy(rp[:, 128 * s:128 * (s + 1)],
                                          ps_tr[:])
                rp_t.append(rp)
                if i == 0:
                    # head-0 half of the logits tail, scheduled under the
                    # head-1 S-phase
                    h0_parts = []
                    for rc in range(4):
                        ps_za = ps_small.tile([128, 1], F32, tag="ps_small",
                                              name=f"ps_za{rc}")
                        nc.tensor.matmul(
                            ps_za[:], rp[HID:XA, 128 * rc:128 * (rc + 1)],
                            ones65[HID:XA, :], start=True, stop=True)
                        rz0 = sbw.tile([128, 1], F32, tag="sm", bufs=8,
                                       name=f"rz0h{rc}")
                        nc.vector.reciprocal(rz0[:], ps_za[:])
                        ps_la = ps_small.tile([128, OUT], F32,
                                              tag="ps_small",
                                              name=f"ps_la{rc}")
                        nc.tensor.matmul(
                            ps_la[:], rp[0:HID, 128 * rc:128 * (rc + 1)],
                            predWb_t[0:HID, :], start=True, stop=True)
                        u = sbw.tile([128, OUT], F32, tag="finh", bufs=4,
                                     name=f"uh{rc}")
                        nc.vector.tensor_scalar(u[:], ps_la[:], rz0[:],
                                                None, op0=ALU.mult)
                        h0_parts.append(u)

            # ---- phase 4: logits + final softmax ----
            ps_pb = ps_small.tile([128, OUT], F32, tag="ps_small", name="ps_pb")
            nc.tensor.matmul(ps_pb[:], ones_row[:], pb_row[:],
                             start=True, stop=True)
            pb_b = sbw.tile([128, OUT], F32, tag="pb_b")
            nc.vector.tensor_copy(pb_b[:], ps_pb[:])
            o_all = sbw.tile([128, 4, OUT], F32, tag="o_all")
            ps_zs, rzs, ps_ls, vs, ws, wbs, es, ss, rss = ([] for _ in range(9))
            for rc in range(4):
                ps_z = ps_pt.tile([128, 1], F32, tag="ps_pt", name=f"ps_z{rc}")
                nc.tensor.matmul(ps_z[:],
                                 rp_t[1][HID:XA, 128 * rc:128 * (rc + 1)],
                                 ones65[HID:XA, :], start=True, stop=True)
                ps_zs.append(ps_z)
                ps_l1 = ps_small.tile([128, OUT], F32, tag="ps_small",
                                      name=f"ps_l1{rc}")
                nc.tensor.matmul(ps_l1[:],
                                 rp_t[1][0:HID, 128 * rc:128 * (rc + 1)],
                                 predWb_t[0:HID, :], start=True, stop=True)
                ps_ls.append(ps_l1)
            for rc in range(4):
                rz = sbw.tile([128, 1], F32, tag="sm", bufs=8, name=f"rz{rc}")
                nc.vector.reciprocal(rz[:], ps_zs[rc][:])
                rzs.append(rz)
            for rc in range(4):
                v = sbw.tile([128, OUT], F32, tag="fv", bufs=4, name=f"v{rc}")
                nc.vector.tensor_scalar(v[:], ps_ls[rc][:], rzs[rc][:], None,
                                        op0=ALU.mult)
                vs.append(v)
            for rc in range(4):
                w_ = sbw.tile([128, OUT], F32, tag="fin", bufs=4, name=f"w{rc}")
                nc.vector.tensor_tensor(w_[:], h0_parts[rc][:], vs[rc][:],
                                        op=ALU.add)
                ws.append(w_)
            for rc in range(4):
                wb = sbw.tile([128, OUT], F32, tag="finh", bufs=4,
                              name=f"wb{rc}")
                nc.vector.tensor_tensor(wb[:], ws[rc][:], pb_b[:], op=ALU.add)
                wbs.append(wb)
            for rc in range(4):
                e = sbw.tile([128, OUT], F32, tag="fe", bufs=4, name=f"e{rc}")
                nc.scalar.activation(e[:], wbs[rc][:], AF.Exp)
                es.append(e)
            for rc in range(4):
                s = sbw.tile([128, 1], F32, tag="sm", bufs=8, name=f"s{rc}")
                nc.vector.reduce_sum(s[:], es[rc][:],
                                     axis=mybir.AxisListType.X)
                ss.append(s)
            for rc in range(4):
                rs = sbw.tile([128, 1], F32, tag="sm", bufs=8, name=f"rs{rc}")
                nc.vector.reciprocal(rs[:], ss[rc][:])
                rss.append(rs)
            for rc in range(4):
                nc.vector.tensor_scalar(o_all[:, rc, :], es[rc][:],
                                        rss[rc][:], None, op0=ALU.mult)
            nc.sync.dma_start(
                out[:, :].rearrange("(rc p) o -> p rc o", rc=4), o_all[:])

    nc.finalize()
    return nc


_NC = None


def _get_nc():
    global _NC
    if _NC is None:
        _NC = build()
    return _NC


def _prepare_in_maps(inputs):
    h = np.asarray(inputs["h"], np.float32)
    adj = np.asarray(inputs["adj"], np.float32)
    W1v = np.asarray(inputs["W1"], np.float32)
    b1v = np.asarray(inputs["b1"], np.float32)

    # shared (replicated) tensors
    hTv = np.ascontiguousarray(h.T.reshape(4, 128, N))
    hThi = hTv.astype(ml_dtypes.bfloat16)
    hTlo = (hTv - hThi.astype(np.float32)).astype(ml_dtypes.bfloat16)
    # W1f[p, ec, 64i+k] = W1[i, 128ec+p, k]
    W1f = np.ascontiguousarray(
        W1v.reshape(H, 4, 128, HID).transpose(2, 1, 0, 3).reshape(128, 4, 2 * HID))
    W1fhi = W1f.astype(ml_dtypes.bfloat16)
    W1flo = (W1f - W1fhi.astype(np.float32)).astype(ml_dtypes.bfloat16)
    b1c = np.ascontiguousarray(b1v.T)  # [64, 2]
    Mt = np.zeros((XA, H, XA), np.float32)
    Wvb1 = np.zeros((XA, H, XA), np.float32)
    for i in range(H):
        Wqb = np.concatenate([np.asarray(inputs["Wq"], np.float32)[i],
                              np.asarray(inputs["bq"], np.float32)[i][None]], 0)
        Wkb = np.concatenate([np.asarray(inputs["Wk"], np.float32)[i],
                              np.asarray(inputs["bk"], np.float32)[i][None]], 0)
        Mt[:, i, :] = Wqb @ Wkb.T
        Wvb = np.concatenate([np.asarray(inputs["Wv"], np.float32)[i],
                              np.asarray(inputs["bv"], np.float32)[i][None]], 0)
        Wvb1[:, i, 0:HID] = Wvb[:, 0:HID]
        Wvb1[HID, i, HID] = 1.0
    Wvb1 = Wvb1.astype(NPF16)
    predWbv = np.ascontiguousarray(np.concatenate(
        [np.asarray(inputs["pred_W"], np.float32),
         np.asarray(inputs["pred_b"], np.float32)[None, :]], axis=0))
    Iden = np.eye(128, dtype=np.float32)

    in_maps = []
    for c in range(W):
        rows = slice(R * c, R * (c + 1))
        # adjTpm[i, p, t, q] = adj[i, Rc+q, 128t+p]
        a = adj[:, rows, :]                       # [H, R, N]
        a = a.transpose(0, 2, 1)                  # [H, N, R] (n, q)
        a = a.reshape(H, 32, 128, R)              # [H, t, p, q]
        a = np.ascontiguousarray(a.transpose(0, 2, 1, 3))  # [H, p, t, q]
        in_maps.append(dict(adjT=a.astype(ml_dtypes.bfloat16),
                            hThi=hThi, hTlo=hTlo, W1fhi=W1fhi, W1flo=W1flo,
                            Mt=Mt, Wvb1=Wvb1, b1=b1c,
                            predWb=predWbv, Iden=Iden))
    return in_maps


def run(inputs, **run_kwargs):
    nc = _get_nc()
    in_maps = _prepare_in_maps(inputs)
    res = bass_utils.run_bass_kernel_spmd(nc, in_maps, core_ids=list(range(W)),
                                          **run_kwargs)
    outp = np.concatenate([res.results[c]["out"] for c in range(W)], axis=0)
    return outp, res


def kernel(**inputs) -> np.ndarray:
    outp, _ = run(inputs)
    return outp
